# revision 43
# baseline (speedup 1.0000x reference)
"""Trainium2 Bass kernel for BiLinearLayerV2.

  biLinear[b,f,d] = sum_e feature[b,f,e] * weight[f,e,d]
  out[b,f,g,d]    = biLinear[b,f,d] * feature[b,g,d] * weightLeft[f,g]

Shapes: feature [512,64,32] f32, weight [64,32,32], weightLeft [64,64]
Output: [512,64,64,32] f32 (256 MB) -> heavily store-bound.

Data-parallel over batch (64 b's per core x 8 cores). Per core the active
scheme is VERSION="v2" (~110 us HW, vs ~195 us for the v1 fused-WV scheme
kept below for reference):

  Stage A (PE, fp32): biLinT_f[d,b] = sum_e weight[f,e,d]*feature[b,f,e];
    64 per-f 32x32 matmuls on 4 diagonal PE tiles (32r,32r) -- diagonal so
    each tile owns its own PSUM column-group write port (4 row tiles
    sharing one column group wedges the device). DVE copies PSUM -> SBUF
    f16 scaled by S1. biLinear stays exact f32: the later f16 factors are
    purely multiplicative, so per-element relative error stays ~1e-3 even
    for cancelling sums (an f16 matmul here blows up the max-rel metric).
  M build (gpsimd, idle otherwise): M_f[d,(g,d')] = (d==d')*wL[f,g]*S2
    from a [128,1024] f16 host mask x broadcast wL tile -- 16 KB of weight
    expanded on-chip instead of the 16 MB fused-WV HBM stream of v1.
    Two half-tiles per j4 group to halve first-availability latency.
  Stage B (PE, f16): pq_f[b,(g,d)] = sum_d' biLinT16_f[d',b]*M_f[d',(g,d)]
    (one nonzero per column); 4 f's concurrent on (32r, 64half) PE tiles.
  DVE: ot = pq * (1/S1/S2) * featd (the 8.4M-elem multiply, ~73 us busy).
  Stores: output DRAM tensor is f-major (row f*64+b) so an f-pair store is
    one contiguous 1 MiB [128,2048] DMA -- the optimal 16-SDMA-engine spray
    shape (a 3-dim strided DRAM AP serializes onto one engine, 7x slower).
    Stores alternate the two HWDGE rings (sync/scalar); the host transposes
    to [b,f,g,d] when unsharding.

Per-core HBM traffic ~35.9 MB (33.6 store + 2.3 load); the kernel runs at
the ~420 GB/s DMA roofline for its whole span minus ~20 us of ramp+tail.
The floor is per-SDMA-engine: 36 MB/16 engines at ~25 GB/s/engine ~= 92 us.
HW exec ~109-126 us across runs (DVFS throttle variance).
"""

import sys

if "/opt/trn_rl_repo" not in sys.path:
    sys.path.insert(0, "/opt/trn_rl_repo")

import numpy as np

B, F, E = 512, 64, 32
NCORES = 8
BLOC = B // NCORES  # 64
GD = F * E  # 2048
SCALE = 1024  # 2^10 pre-scale keeps fp16 operand values out of the subnormal range

# dtype of the Q-matmul operands:
#   "f32"   exact (rel err ~4e-7), fp32 streams at 4 cyc/col
#   "f16"   fastest (~1e-3 max rel err), 1 cyc/col, halves WV DMA
#   "f16x2" fp16 hi+lo split of feature, two accumulating matmuls:
#           feature exact, only WV rounds -> ~5e-4 max rel err
QDT = "f32"

# VERSION "v2": two-stage PE scheme that avoids the 16 MB WV load entirely.
#   Stage A (PE, fp32): biLinT_f[d,b] = sum_e weight[f,e,d] * feature[b,f,e]
#     64 per-f 32x32 matmuls, 16 concurrent via (32r,32c) tile_position.
#     DVE copies PSUM -> SBUF as f16 scaled by S1=1024.
#   M build (gpsimd): M_f[d,(g,d')] = (d==d') * wL[f,g]*S2, from a host mask
#     [128,2048] f16 and a wL broadcast tile -- 16 KB of weight data expanded
#     on-chip instead of 16 MB streamed from HBM.
#   Stage B (PE, f16): pq_f[b,(g,d)] = sum_d' biLinT16_f[d',b] * M_f[d',(g,d)]
#     = biLin*wL*S1*S2 (single nonzero per column), same geometry as v1.
#   DVE: ot = pq * (1/S1/S2) * featd;  stores are 1 MiB / 128-partition
#   (f-pair-major DRAM AP) round-robined over sync/scalar/gpsimd rings.
# Per-core HBM traffic: ~2.3 MB reads + 33.6 MB writes (vs 20 + 33.6 in v1).
VERSION = "v2"
S1 = 1024.0  # biLinT f16 scale
S2 = 1024.0  # weightLeft f16 scale
# Bisection flags for the v2 kernel's riskier pieces:
BIG_STORE = True  # 1 MiB f-pair stores via transposed DRAM AP (vs v1-style)
MBUILD_ENGINE = "gpsimd"  # engine for the M-tile builds ("gpsimd"/"vector")
STORE_ENGS = 2  # stores round-robin over first N of (sync, scalar, gpsimd)
OT_BUFS = 8  # store-side SBUF queue depth

_cached = {}


def _build_nc(qdt=QDT):
    from contextlib import ExitStack

    import concourse.bass as bass
    import concourse.tile as tile
    from concourse import bacc, mybir

    f32 = mybir.dt.float32
    qdt_my = {
        "f32": mybir.dt.float32,
        "f16": mybir.dt.float16,
        "f16x2": mybir.dt.float16,
    }[qdt]
    fsplit = qdt == "f16x2"
    scaled = qdt in ("f16", "f16x2")
    nterms = 2 if fsplit else 1
    nc = bacc.Bacc("TRN2", target_bir_lowering=False, debug=False)

    featd = nc.dram_tensor("featd", (2 * BLOC, GD), f32, kind="ExternalInput").ap()
    featT = nc.dram_tensor(
        "featT", (128, F * BLOC), qdt_my, kind="ExternalInput"
    ).ap()
    if fsplit:
        featT_lo = nc.dram_tensor(
            "featT_lo", (128, F * BLOC), qdt_my, kind="ExternalInput"
        ).ap()
    wv = nc.dram_tensor("wv", (16, 128, GD), qdt_my, kind="ExternalInput").ap()
    out = nc.dram_tensor("out", (BLOC, F, GD), f32, kind="ExternalOutput").ap()

    with tile.TileContext(nc) as tc, ExitStack() as ctx:
        consts = ctx.enter_context(tc.tile_pool(name="consts", bufs=1))
        featd_t = consts.tile([128, GD], f32)
        nc.scalar.dma_start(featd_t[:], featd)
        featT_t = consts.tile([128, F * BLOC], qdt_my)
        nc.scalar.dma_start(featT_t[:], featT)
        if fsplit:
            featT_lo_t = consts.tile([128, F * BLOC], qdt_my)
            nc.scalar.dma_start(featT_lo_t[:], featT_lo)

        with (
            tc.tile_pool(name="wd", bufs=4) as wdp,
            tc.tile_pool(name="psq", bufs=1, space=bass.MemorySpace.PSUM) as psq,
            tc.tile_pool(name="ot", bufs=6) as otp,
        ):
            for j4 in range(16):
                wdt = wdp.tile([128, GD], qdt_my)
                nc.gpsimd.dma_start(wdt[:], wv[j4])
                # Both s-pairs' matmuls interleaved at term level so all four
                # disjoint PE row/col regions (r=0..3) run concurrently.
                pqs = [
                    psq.tile([128, GD], f32, name=f"pq{s}", tag=f"pq{s}")
                    for s in range(2)
                ]
                for n in range(4):
                    nsl = slice(512 * n, 512 * (n + 1))
                    for s in range(2):
                        for half in range(2):
                            r = 2 * s + half
                            f = 4 * j4 + r
                            rsl = slice(32 * r, 32 * r + 32)
                            fsl = slice(f * BLOC, (f + 1) * BLOC)
                            for t in range(nterms):
                                src = featT_t if t == 0 else featT_lo_t
                                nc.tensor.matmul(
                                    pqs[s][64 * half : 64 * half + 64, nsl],
                                    src[rsl, fsl],
                                    wdt[rsl, nsl],
                                    start=(t == 0),
                                    stop=(t == nterms - 1),
                                    tile_position=(32 * r, 64 * half),
                                )
                for s in range(2):
                    pq = pqs[s]
                    ot = otp.tile([128, GD], f32)
                    if scaled:
                        # out = (Q / S^2) * feature, undoing the fp16 scaling
                        nc.vector.scalar_tensor_tensor(
                            ot[:],
                            pq[:],
                            1.0 / float(SCALE * SCALE),
                            featd_t[:],
                            op0=mybir.AluOpType.mult,
                            op1=mybir.AluOpType.mult,
                        )
                    else:
                        nc.vector.tensor_mul(ot[:], pq[:], featd_t[:])
                    f0 = 4 * j4 + 2 * s
                    eng = nc.sync if s == 0 else nc.scalar
                    eng.dma_start(out[:, f0, :], ot[0:64, :])
                    eng.dma_start(out[:, f0 + 1, :], ot[64:128, :])

    nc.compile()
    return nc


def _build_nc_v2():
    from contextlib import ExitStack

    import concourse.bass as bass
    import concourse.tile as tile
    from concourse import bacc, mybir

    f32 = mybir.dt.float32
    f16 = mybir.dt.float16
    mult = mybir.AluOpType.mult
    nc = bacc.Bacc("TRN2", target_bir_lowering=False, debug=False)

    featT = nc.dram_tensor("featT32", (128, 1024), f32, kind="ExternalInput").ap()
    wt = nc.dram_tensor("wt32", (128, 512), f32, kind="ExternalInput").ap()
    featd = nc.dram_tensor("featd", (BLOC, GD), f32, kind="ExternalInput").ap()
    wLrep = nc.dram_tensor("wLrep16", (128, 1024), f16, kind="ExternalInput").ap()
    mask = nc.dram_tensor("mask16", (128, GD // 2), f16, kind="ExternalInput").ap()
    # f-major output: row f*BLOC+b. An f-pair store is then one contiguous
    # 1 MiB [128, 2048] DMA (the optimal 16-engine spray shape); the host
    # untransposes to [b, f, g, d] when unsharding.
    out = nc.dram_tensor("out", (F * BLOC, GD), f32, kind="ExternalOutput").ap()

    with tile.TileContext(nc) as tc, ExitStack() as ctx:
        consts = ctx.enter_context(tc.tile_pool(name="consts", bufs=1))
        # Ramp-critical loads: mask/wLrep first (they gate the gpsimd M-build
        # chain, the ramp's pacing item), then stage-A inputs, featd last.
        mask_t = consts.tile([128, GD // 2], f16)
        nc.sync.dma_start(mask_t[:], mask)
        wLrep_t = consts.tile([128, 1024], f16)
        nc.scalar.dma_start(wLrep_t[:], wLrep)
        wt_t = consts.tile([128, 512], f32)
        nc.sync.dma_start(wt_t[:], wt)
        featT_tiles = [
            consts.tile([128, 256], f32, name=f"fT{p}", tag=f"fT{p}")
            for p in range(4)
        ]
        for p in range(4):
            nc.scalar.dma_start(
                featT_tiles[p][:], featT[:, 256 * p : 256 * (p + 1)]
            )
        featd_t = consts.tile([128, GD], f32)
        nc.sync.dma_start(featd_t[0:64, :], featd)
        nc.scalar.dma_start(featd_t[64:128, :], featd)

        biLinT_tiles = [
            consts.tile([128, 256], f16, name=f"bL{p}", tag=f"bL{p}")
            for p in range(4)
        ]
        # M_f[32r+d, (g,d')] = mask(d,d') * wL16[4j4+r, g], built on the
        # otherwise-idle gpsimd as two half-tiles per j4 (halves the latency
        # until stage B's first n-chunks can run).
        m_tiles = [
            [
                consts.tile([128, 1024], f16, name=f"M{j4}{h}", tag=f"M{j4}{h}")
                for h in range(2)
            ]
            for j4 in range(16)
        ]
        for j4 in range(16):
            for h in range(2):
                mv = mask_t[:].rearrange("p (g dp) -> p g dp", dp=E)
                wvb = wLrep_t[
                    :, 64 * j4 + 32 * h : 64 * j4 + 32 * (h + 1), None
                ].to_broadcast([128, 32, E])
                nc.gpsimd.tensor_tensor(
                    m_tiles[j4][h][:].rearrange("p (g dp) -> p g dp", dp=E),
                    mv,
                    wvb,
                    op=mult,
                )

        # Stage A: biLinT[32r+d, 64(j4%4)+b] in quad tile j4//4, f = 4j4+r.
        # Four diagonal 32x32 PE tiles (32r,32r) per pass -- each tile owns
        # its own PSUM column group (write port), matching the v1 pattern.
        with tc.tile_pool(name="psa", bufs=1, space=bass.MemorySpace.PSUM) as psap:
            psa_tiles = [
                psap.tile([128, 256], f32, name=f"psa{p}", tag=f"psa{p}")
                for p in range(4)
            ]
            for p in range(4):
                for jl in range(4):
                    j4 = 4 * p + jl
                    for r in range(4):
                        nc.tensor.matmul(
                            psa_tiles[p][32 * r : 32 * r + 32, 64 * jl : 64 * jl + 64],
                            wt_t[32 * r : 32 * r + 32, 32 * j4 : 32 * j4 + 32],
                            featT_tiles[p][32 * r : 32 * r + 32, 64 * jl : 64 * jl + 64],
                            start=True,
                            stop=True,
                            tile_position=(32 * r, 32 * r),
                        )
                nc.vector.tensor_scalar_mul(biLinT_tiles[p][:], psa_tiles[p][:], S1)

        # Stage B + output multiply + 1 MiB stores
        with (
            tc.tile_pool(name="psq", bufs=1, space=bass.MemorySpace.PSUM) as psq,
            tc.tile_pool(name="ot", bufs=OT_BUFS) as otp,
        ):
            for j4 in range(16):
                bL = biLinT_tiles[j4 // 4]
                csl = slice(64 * (j4 % 4), 64 * (j4 % 4) + 64)
                pqs = [
                    psq.tile([128, GD], f32, name=f"pq{s}", tag=f"pq{s}")
                    for s in range(2)
                ]
                for n in range(4):
                    nsl = slice(512 * n, 512 * (n + 1))
                    mh = m_tiles[j4][n // 2]
                    msl = slice(512 * (n % 2), 512 * (n % 2) + 512)
                    for s in range(2):
                        for half in range(2):
                            r = 2 * s + half
                            nc.tensor.matmul(
                                pqs[s][64 * half : 64 * half + 64, nsl],
                                bL[32 * r : 32 * r + 32, csl],
                                mh[32 * r : 32 * r + 32, msl],
                                start=True,
                                stop=True,
                                tile_position=(32 * r, 64 * half),
                            )
                for s in range(2):
                    pq = pqs[s]
                    ot = otp.tile([128, GD], f32)
                    nc.vector.scalar_tensor_tensor(
                        ot[:],
                        pq[:],
                        1.0 / (S1 * S2),
                        featd_t[:],
                        op0=mult,
                        op1=mult,
                    )
                    f0 = 4 * j4 + 2 * s
                    eng = (nc.sync, nc.scalar, nc.gpsimd)[(2 * j4 + s) % STORE_ENGS]
                    if BIG_STORE:
                        eng.dma_start(out[f0 * BLOC : (f0 + 2) * BLOC, :], ot[:])
                    else:
                        eng.dma_start(out[f0 * BLOC : (f0 + 1) * BLOC, :], ot[0:64, :])
                        eng.dma_start(
                            out[(f0 + 1) * BLOC : (f0 + 2) * BLOC, :], ot[64:128, :]
                        )

    nc.compile()
    return nc


def _get_nc(qdt=QDT):
    key = "v2" if VERSION == "v2" else qdt
    if key not in _cached:
        _cached[key] = _build_nc_v2() if key == "v2" else _build_nc(qdt)
    return _cached[key]


def _host_inputs_v2(feature, weight, weightLeft):
    """v2 per-core input maps: tiny raw-weight layouts, no WV expansion."""
    feature = np.ascontiguousarray(feature, dtype=np.float32)
    weight = np.ascontiguousarray(weight, dtype=np.float32)
    wL = np.ascontiguousarray(weightLeft, dtype=np.float32)

    # Band r serves f = 4*j4 + r (f % 4 == r), one j4 column-block per pass.
    # wt32[32r+e, 32j4+d] = weight[4j4+r, e, d]
    wt32 = np.empty((4, 32, 16, 32), dtype=np.float32)
    for r in range(4):
        wt32[r] = weight[4 * np.arange(16) + r].transpose(1, 0, 2)
    wt32 = np.ascontiguousarray(wt32.reshape(128, 512))

    # wLrep16[32r+d, 64j4+g] = wL[4j4+r, g] * S2
    wlr = wL.reshape(16, 4, F).transpose(1, 0, 2)  # [r, j4, g]
    wLrep16 = np.ascontiguousarray(
        np.broadcast_to((wlr * np.float32(S2)).astype(np.float16)[:, None, :, :],
                        (4, 32, 16, F)).reshape(128, 1024)
    )

    # mask16[32r+d, 32g+d'] = (d == d')
    eye = np.eye(E, dtype=np.float16)
    band = np.broadcast_to(eye[:, None, :], (E, F // 2, E)).reshape(E, GD // 2)
    mask16 = np.ascontiguousarray(np.tile(band, (4, 1)))

    in_maps = []
    for c in range(NCORES):
        fc = feature[c * BLOC : (c + 1) * BLOC]  # [64, 64, 32]
        featd = np.ascontiguousarray(fc.reshape(BLOC, GD))
        # featT32[32r+e, 64j4+b] = fc[b, 4j4+r, e]
        ft = np.empty((4, 32, 16, BLOC), dtype=np.float32)
        for r in range(4):
            ft[r] = fc[:, 4 * np.arange(16) + r, :].transpose(2, 1, 0)
        in_maps.append(
            {
                "featT32": np.ascontiguousarray(ft.reshape(128, 1024)),
                "wt32": wt32.copy(),
                "featd": featd,
                "wLrep16": wLrep16.copy(),
                "mask16": mask16.copy(),
            }
        )
    return in_maps


def _host_inputs_v1(feature, weight, weightLeft, qdt=QDT):
    """Per-core input maps. Host work is layout prep of weights/inputs only."""
    feature = np.ascontiguousarray(feature, dtype=np.float32)
    weight = np.ascontiguousarray(weight, dtype=np.float32)
    weightLeft = np.ascontiguousarray(weightLeft, dtype=np.float32)

    # WV[f, e', g, d] = weight[f,e',d] * W[f,g], fused in fp64, grouped so
    # partitions 32r..32r+31 of group j4 hold WV for f = 4*j4 + r.
    wv64 = weight.astype(np.float64)[:, :, None, :] * weightLeft.astype(np.float64)[
        :, None, :, None
    ]  # [F, E, F(g), E(d)]
    if qdt == "f32":
        wv = wv64.astype(np.float32).reshape(16, 4 * E, F * E)
    else:
        wv = (wv64 * SCALE).astype(np.float32).astype(np.float16)
        wv = wv.reshape(16, 4 * E, F * E)
    wv = np.ascontiguousarray(wv)

    in_maps = []
    for c in range(NCORES):
        fc = feature[c * BLOC : (c + 1) * BLOC]  # [64, 64, 32]
        featd = np.ascontiguousarray(
            np.tile(fc.reshape(BLOC, GD), (2, 1))
        )  # [128, 2048]
        ft = np.ascontiguousarray(fc.transpose(2, 1, 0)).reshape(E, F * BLOC)
        m = {"featd": featd}
        if qdt == "f32":
            m["featT"] = np.ascontiguousarray(np.tile(ft, (4, 1)))  # [128, 4096]
        else:
            fts = ft * np.float32(SCALE)
            hi = fts.astype(np.float16)
            m["featT"] = np.ascontiguousarray(np.tile(hi, (4, 1)))
            if qdt == "f16x2":
                lo = (fts - hi.astype(np.float32)).astype(np.float16)
                m["featT_lo"] = np.ascontiguousarray(np.tile(lo, (4, 1)))
        m["wv"] = wv
        in_maps.append(m)
    return in_maps


def _host_inputs(feature, weight, weightLeft, qdt=QDT):
    if VERSION == "v2":
        return _host_inputs_v2(feature, weight, weightLeft)
    return _host_inputs_v1(feature, weight, weightLeft, qdt)


def _run(in_maps, trace=False, tmpdir=None, qdt=QDT):
    from concourse.bass_utils import run_bass_kernel_spmd

    nc = _get_nc(qdt)
    return run_bass_kernel_spmd(
        nc, in_maps, core_ids=list(range(NCORES)), trace=trace, tmpdir=tmpdir
    )


def _unshard(results):
    """Per-core device outputs -> full [B, F, F, E] array."""
    if VERSION == "v2":
        # v2 device layout is f-major: out[f*BLOC+b, (g,d)]
        return np.concatenate(
            [
                r["out"].reshape(F, BLOC, F, E).transpose(1, 0, 2, 3)
                for r in results
            ],
            axis=0,
        )
    return np.concatenate(
        [r["out"].reshape(BLOC, F, F, E) for r in results], axis=0
    )


def kernel(feature, weight, weightLeft):
    in_maps = _host_inputs(feature, weight, weightLeft)
    res = _run(in_maps)
    return _unshard(res.results)

